# revision 1
# baseline (speedup 1.0000x reference)
"""Trainium2 Bass kernel for the CMIFE module (nn_CMIFE_1314259993166).

Pure data parallel: 1 sample per NeuronCore (8 cores, batch 8).

Layout notes (per core):
  - x, out DRAM tensors are f16 [C, HW]; kernel() converts on the host.
  - aligned activations stay SBUF-resident f16 as 2 blocks of [128, HW].
  - pass B transposes ca-scaled aligned chunks with an extra "avg" column
    (mean <= max, so the avg column needs no masking; the max reduce uses
    an AP view that skips it anyway) giving sf in chunk layout, then PE
    transposes assemble pixel-order sf for DRAM (sfe + sf_cm_pad).
  - grid sample: one gpsimd ap_gather per core group; the 16 partitions of
    each gpsimd core hold a per-group source window at pixel shifts
    {0,+1,+W,+W+1} so a single gather yields all 4 bilinear corners.
"""

import numpy as np

import concourse.bacc as bacc
import concourse.bass as bass
import concourse.mybir as mybir
from concourse.masks import make_identity
from concourse.tile import TileContext

dt = mybir.dt
OP = mybir.AluOpType
AF = mybir.ActivationFunctionType
AX = mybir.AxisListType
F32, F16, I32, I16 = dt.float32, dt.float16, dt.int32, dt.int16

# ---- problem constants (hardcoded per contract) ----
B = 8
C = 256
H = W = 160
HW = H * W                    # 25600
MID = 16
EPS = 1e-5
PW, PH = W + 2, H + 2         # 162 (3x3-pad buffers)
PHW = PH * PW                 # 26244
P7W, P7H = W + 6, H + 6       # 166 (7x7-pad buffer)
P7HW = P7H * P7W              # 27556

SLAB = 5120                   # 32 rows
NSLAB = HW // SLAB            # 5
HALF = HW // 2                # 12800 (80 rows) - conv im2col span
QTR = HW // 4                 # 6400 (40 rows) - conv7 span
CH7 = 400                     # 7x7 conv pixel chunk

# gather window geometry: 8 gpsimd cores x 20-row bands, +/-5 row margin
GT = 5                        # sfe top guard rows
GB = 9                        # sfe bottom guard rows
SFE_ROWS = H + GT + GB        # 174
SFE_NE = SFE_ROWS * W         # 27840
WROWS = 32                    # window rows per group
WNE = WROWS * W               # 5120 window pixels
IDX_MAX = WNE - 1 - (W + 1)   # safety clamp for gather indices


def build(debug=False):
    nc = bacc.Bacc("TRN2", target_bir_lowering=False, debug=False, num_devices=B)

    P = {}
    P['x'] = nc.dram_tensor('x', [C, HW], F16, kind="ExternalInput").ap()
    P['align_w'] = nc.dram_tensor('align_w', [C, C], F32, kind="ExternalInput").ap()
    for n in ('align_g', 'align_b', 'align_m', 'align_v'):
        P[n] = nc.dram_tensor(n, [1, C], F32, kind="ExternalInput").ap()
    P['mlp_w1'] = nc.dram_tensor('mlp_w1', [MID, C], F32, kind="ExternalInput").ap()
    P['mlp_w2'] = nc.dram_tensor('mlp_w2', [C, MID], F32, kind="ExternalInput").ap()
    P['loc_w1'] = nc.dram_tensor('loc_w1', [MID, C], F32, kind="ExternalInput").ap()
    P['loc_w2'] = nc.dram_tensor('loc_w2', [C, MID], F32, kind="ExternalInput").ap()
    P['fusion_w'] = nc.dram_tensor('fusion_w', [1, 1], F32, kind="ExternalInput").ap()
    P['off_w1'] = nc.dram_tensor('off_w1', [MID, 18], F32, kind="ExternalInput").ap()
    for n in ('off_g', 'off_bt', 'off_m', 'off_v'):
        P[n] = nc.dram_tensor(n, [1, MID], F32, kind="ExternalInput").ap()
    P['off_w2'] = nc.dram_tensor('off_w2', [98, 144], F32, kind="ExternalInput").ap()
    P['off_b2'] = nc.dram_tensor('off_b2', [1, 98], F32, kind="ExternalInput").ap()
    P['attn_w'] = nc.dram_tensor('attn_w', [1, 98], F32, kind="ExternalInput").ap()
    P['out'] = nc.dram_tensor('out', [C, HW], F16, kind="ExternalOutput").ap()

    # DRAM scratch
    P['sf_cm_pad'] = nc.dram_tensor('sf_cm_pad', [2, PHW + 4], F16).ap()
    P['sf_cm'] = nc.dram_tensor('sf_cm', [2, HW], F16).ap()
    P['samp_cm'] = nc.dram_tensor('samp_cm', [2, HW], F16).ap()
    P['off_cm'] = nc.dram_tensor('off_cm', [2, HW], F16).ap()
    P['sfe'] = nc.dram_tensor('sfe', [SFE_NE, 2], F16).ap()
    P['o1_pad'] = nc.dram_tensor('o1_pad', [MID, PHW + 4], F16).ap()
    P['samp_pad'] = nc.dram_tensor('samp_pad', [2, P7HW + 8], F16).ap()

    P['dbg'] = {}
    if debug:
        for name, shape, ddt in [
                ('d_sf', [2, HW], F16), ('d_ca', [C, 1], F32),
                ('d_off', [2, HW], F16), ('d_samp', [2, HW], F16),
                ('d_sa', [1, HW], F16)]:
            P['dbg'][name] = nc.dram_tensor(name, shape, ddt,
                                            kind="ExternalOutput").ap()

    with TileContext(nc) as tc:
        _body(nc, tc, P)
    nc.compile()
    return nc


def _tile(pool, shape, dtype, tag):
    return pool.tile(shape, dtype, tag=tag, name=tag)


def _safe_floor(nc, pool, v, shape, tag):
    """floor(v) robust to cast rounding mode (trunc on sim, rtn on hw)."""
    vi = _tile(pool, shape, I32, f'{tag}_i')
    nc.vector.tensor_copy(vi[:], v[:])
    vf = _tile(pool, shape, F32, f'{tag}_f')
    nc.vector.tensor_copy(vf[:], vi[:])
    d = _tile(pool, shape, F32, f'{tag}_d')
    nc.vector.tensor_tensor(out=d[:], in0=vf[:], in1=v[:], op=OP.is_gt)
    nc.vector.tensor_tensor(out=vf[:], in0=vf[:], in1=d[:], op=OP.subtract)
    return vf


def _body(nc, tc, P):
    dbg = P['dbg']
    x, out = P['x'], P['out']

    cpool = tc.alloc_tile_pool(name='const', bufs=1)
    apool = tc.alloc_tile_pool(name='aligned', bufs=1)

    aligned = [_tile(apool, [128, HW], F16, 'a0'),
               _tile(apool, [128, HW], F16, 'a1')]

    ident = _tile(cpool, [128, 128], F32, 'ident')
    make_identity(nc, ident[:])
    ident16 = _tile(cpool, [128, 128], F16, 'ident16')
    nc.vector.tensor_copy(ident16[:], ident[:])
    ones1 = _tile(cpool, [1, 128], F16, 'ones1')
    nc.vector.memset(ones1[:], 1.0)
    zeros = _tile(cpool, [MID, 512], F16, 'zeros')
    nc.vector.memset(zeros[:], 0.0)

    # ---- zero borders of padded DRAM buffers ----
    sf_cm_pad, o1_pad, samp_pad, sfe = (P['sf_cm_pad'], P['o1_pad'],
                                        P['samp_pad'], P['sfe'])
    sf_cm = P['sf_cm']
    for buf, nch, w_, h_, pad, cst in ((sf_cm_pad, 2, PW, PH, 1, PHW + 4),
                                       (o1_pad, MID, PW, PH, 1, PHW + 4),
                                       (samp_pad, 2, P7W, P7H, 3, P7HW + 8)):
        nc.gpsimd.dma_start(out=buf[0:nch, 0:pad * w_], in_=zeros[0:nch, 0:pad * w_])
        nc.gpsimd.dma_start(out=buf[0:nch, (h_ - pad) * w_:h_ * w_],
                            in_=zeros[0:nch, 0:pad * w_])
        for off in (pad * w_, pad * w_ + w_ - pad):
            col = bass.AP(buf.tensor, off, [[cst, nch],
                                            [w_, h_ - 2 * pad], [1, pad]])
            nc.gpsimd.dma_start(out=col, in_=zeros[0:nch, 0:(h_ - 2 * pad) * pad])
    nc.gpsimd.dma_start(out=sf_cm_pad[0:2, PHW:PHW + 4], in_=zeros[0:2, 0:4])
    nc.gpsimd.dma_start(out=o1_pad[0:MID, PHW:PHW + 4], in_=zeros[0:MID, 0:4])
    nc.gpsimd.dma_start(out=samp_pad[0:2, P7HW:P7HW + 8], in_=zeros[0:2, 0:8])
    # sfe guard rows (top GT rows, bottom GB rows)
    nc.gpsimd.dma_start(out=sfe[0:GT * W, :], in_=zeros[0:4, 0:GT * W // 2])
    nc.gpsimd.dma_start(out=sfe[(GT + H) * W:SFE_NE, :],
                        in_=zeros[0:6, 0:GB * W // 3])

    # ================= weight prep =================
    wprep = tc.alloc_tile_pool(name='wprep', bufs=1)
    wpp = tc.alloc_tile_pool(name='wprep_ps', bufs=2, space="PSUM")

    def bn_fold(gv, bv, mv, vv, n, pfx):
        t = {}
        for nm, a in (('g', gv), ('b', bv), ('m', mv), ('v', vv)):
            t[nm] = _tile(wprep, [1, n], F32, f'{pfx}{nm}')
            nc.sync.dma_start(out=t[nm][:], in_=a)
        sc = _tile(wprep, [1, n], F32, f'{pfx}sc')
        bi = _tile(wprep, [1, n], F32, f'{pfx}bi')
        nc.vector.tensor_scalar_add(sc[:], t['v'][:], EPS)
        nc.scalar.sqrt(sc[:], sc[:])
        nc.vector.reciprocal(sc[:], sc[:])
        nc.vector.tensor_tensor(out=sc[:], in0=t['g'][:], in1=sc[:], op=OP.mult)
        nc.vector.tensor_tensor(out=bi[:], in0=t['m'][:], in1=sc[:], op=OP.mult)
        nc.vector.tensor_tensor(out=bi[:], in0=t['b'][:], in1=bi[:], op=OP.subtract)
        return sc, bi

    asc_row, abi_row = bn_fold(P['align_g'], P['align_b'], P['align_m'],
                               P['align_v'], C, 'aln')
    aln_sc, aln_bi = [], []
    for b in range(2):
        sct = _tile(cpool, [128, 1], F32, f'asc{b}')
        bit = _tile(cpool, [128, 1], F32, f'abi{b}')
        nc.sync.dma_start(out=sct[:], in_=asc_row[0:1, b * 128:(b + 1) * 128])
        nc.sync.dma_start(out=bit[:], in_=abi_row[0:1, b * 128:(b + 1) * 128])
        aln_sc.append(sct)
        aln_bi.append(bit)

    # align_w^T fp16 tiles (rows pre-scaled by the BN scale)
    wT = [[None, None], [None, None]]
    wsb = [_tile(wprep, [128, C], F32, f'wsb{i}') for i in range(2)]
    nc.sync.dma_start(out=wsb[0][:], in_=P['align_w'][0:128, :])
    nc.sync.dma_start(out=wsb[1][:], in_=P['align_w'][128:256, :])
    for i in range(2):
        nc.vector.tensor_scalar_mul(wsb[i][:], wsb[i][:], aln_sc[i][:])
    for kb in range(2):
        for mb in range(2):
            ps = _tile(wpp, [128, 128], F32, 'wp')
            nc.tensor.transpose(out=ps[:], in_=wsb[mb][:, kb * 128:(kb + 1) * 128],
                                identity=ident[:])
            t16 = _tile(cpool, [128, 128], F16, f'wT{kb}{mb}')
            nc.vector.tensor_copy(t16[:], ps[:])
            wT[kb][mb] = t16

    def load_mlp(w1_ap, w2_ap, pfx):
        w1sb = _tile(wprep, [MID, C], F32, f'{pfx}w1sb')
        nc.sync.dma_start(out=w1sb[:], in_=w1_ap)
        w1T = []
        for b in range(2):
            ps = _tile(wpp, [128, MID], F32, 'wp')
            nc.tensor.transpose(out=ps[:], in_=w1sb[:, b * 128:(b + 1) * 128],
                                identity=ident[0:MID, 0:MID])
            t16 = _tile(cpool, [128, MID], F16, f'{pfx}w1T{b}')
            nc.vector.tensor_copy(t16[:], ps[:])
            w1T.append(t16)
        w2sb = _tile(wprep, [128, 2 * MID], F32, f'{pfx}w2sb')
        nc.sync.dma_start(out=w2sb[:],
                          in_=bass.AP(w2_ap.tensor, 0, [[MID, 128], [128 * MID, 2],
                                                        [1, MID]]))
        w2T = []
        for b in range(2):
            ps = _tile(wpp, [MID, 128], F32, 'wp')
            nc.tensor.transpose(out=ps[:], in_=w2sb[:, b * MID:(b + 1) * MID],
                                identity=ident[:])
            t16 = _tile(cpool, [MID, 128], F16, f'{pfx}w2T{b}')
            nc.vector.tensor_copy(t16[:], ps[:])
            w2T.append(t16)
        return w1T, w2T

    mlp_w1T, mlp_w2T = load_mlp(P['mlp_w1'], P['mlp_w2'], 'mlp')
    loc_w1T, loc_w2T = load_mlp(P['loc_w1'], P['loc_w2'], 'loc')

    # off conv1 lhsT [18, 16], rows reordered (cin,dy,dx)->(dy,cin,dx)
    ow1sb = _tile(wprep, [MID, 18], F32, 'ow1sb')
    nc.sync.dma_start(out=ow1sb[:], in_=P['off_w1'])
    ow1r = _tile(wprep, [MID, 18], F32, 'ow1r')
    src_r = bass.AP(ow1sb.tensor, ow1sb[:].offset,
                    [ow1sb[:].ap[0], [3, 3], [9, 2], [1, 3]])
    nc.vector.tensor_copy(ow1r[:].rearrange("p (a b c) -> p a b c", a=3, b=2), src_r)
    ps = _tile(wpp, [18, MID], F32, 'wp')
    nc.tensor.transpose(out=ps[:], in_=ow1r[:, :], identity=ident[0:MID, 0:MID])
    ow1T = _tile(cpool, [18, MID], F16, 'ow1T')
    nc.vector.tensor_copy(ow1T[:], ps[:])

    # off conv2: collapse 98->2 (group mean) with rows (dy,cin,dx), 64-aligned
    ow2sb = _tile(wprep, [98, 144], F32, 'ow2sb')
    nc.sync.dma_start(out=ow2sb[:], in_=P['off_w2'])
    ow2r = _tile(wprep, [98, 192], F16, 'ow2r')
    nc.vector.memset(ow2r[:], 0.0)
    for dy in range(3):
        src_d = bass.AP(ow2sb.tensor, ow2sb[:].offset + 3 * dy,
                        [ow2sb[:].ap[0], [9, MID], [1, 3]])
        nc.vector.tensor_copy(
            ow2r[:, dy * 64:dy * 64 + 48].rearrange("p (b c) -> p b c", b=MID), src_d)
    indic = _tile(wprep, [98, 2], F16, 'indic')
    pidx = _tile(wprep, [98, 1], I32, 'pidx')
    nc.gpsimd.iota(pidx[:], pattern=[[0, 1]], base=0, channel_multiplier=1)
    pidf = _tile(wprep, [98, 1], F32, 'pidf')
    nc.vector.tensor_copy(pidf[:], pidx[:])
    ind0 = _tile(wprep, [98, 1], F32, 'ind0')
    nc.vector.tensor_scalar(ind0[:], pidf[:], 48.5, 1.0 / 49.0, OP.is_lt, OP.mult)
    nc.vector.tensor_copy(indic[:, 0:1], ind0[:])
    nc.vector.tensor_scalar(ind0[:], ind0[:], -1.0, 1.0 / 49.0, OP.mult, OP.add)
    nc.vector.tensor_copy(indic[:, 1:2], ind0[:])
    ps_a = _tile(wpp, [128, 2], F32, 'wp')
    nc.tensor.matmul(ps_a[:], lhsT=ow2r[:, 0:128], rhs=indic[:], start=True, stop=True)
    ps_b = _tile(wpp, [64, 2], F32, 'wp')
    nc.tensor.matmul(ps_b[:], lhsT=ow2r[:, 128:192], rhs=indic[:], start=True, stop=True)
    w2effT = []
    for dy, (src_ps, lo) in enumerate(((ps_a, 0), (ps_a, 64), (ps_b, 0))):
        t16 = _tile(cpool, [48, 2], F16, f'w2effT{dy}')
        nc.vector.tensor_copy(t16[:], src_ps[lo:lo + 48, :])
        w2effT.append(t16)
    # b2eff [2, 1]
    ob2 = _tile(wprep, [1, 98], F32, 'ob2')
    nc.sync.dma_start(out=ob2[:], in_=P['off_b2'])
    ob2c = _tile(wprep, [98, 1], F16, 'ob2c')
    ob2r = _tile(wprep, [1, 98], F16, 'ob2r')
    nc.vector.tensor_copy(ob2r[:], ob2[:])
    nc.sync.dma_start(out=ob2c[:], in_=ob2r[:])
    ps_b2 = _tile(wpp, [1, 2], F32, 'wp')
    nc.tensor.matmul(ps_b2[:], lhsT=ob2c[:], rhs=indic[:], start=True, stop=True)
    b2row = _tile(wprep, [1, 2], F32, 'b2row')
    nc.vector.tensor_copy(b2row[:], ps_b2[:])
    b2eff = _tile(cpool, [2, 1], F32, 'b2eff')
    nc.sync.dma_start(out=b2eff[:], in_=b2row[:])

    # attn 7x7 lhsT [98, 128] (same weight replicated along free dim)
    awsb = _tile(wprep, [1, 98], F32, 'awsb')
    nc.sync.dma_start(out=awsb[:], in_=P['attn_w'])
    awr = _tile(wprep, [1, 98], F16, 'awr')
    src_a = bass.AP(awsb.tensor, awsb[:].offset, [awsb[:].ap[0], [7, 7], [49, 2], [1, 7]])
    nc.vector.tensor_copy(awr[:].rearrange("p (a b c) -> p a b c", a=7, b=2), src_a)
    attnT = _tile(wprep, [98, 1], F16, 'attnT')
    nc.sync.dma_start(out=attnT[:], in_=awr[:])
    attn_rep = _tile(cpool, [98, 128], F16, 'attn_rep')
    rep_src = bass.AP(attnT.tensor, attnT[:].offset, [attnT[:].ap[0], [0, 128]])
    nc.vector.tensor_copy(attn_rep[:], rep_src)

    osc_row, obi_row = bn_fold(P['off_g'], P['off_bt'], P['off_m'], P['off_v'],
                               MID, 'off')
    off_sc = _tile(cpool, [MID, 1], F32, 'offsc')
    off_bi = _tile(cpool, [MID, 1], F32, 'offbi')
    nc.sync.dma_start(out=off_sc[:], in_=osc_row[0:1, :])
    nc.sync.dma_start(out=off_bi[:], in_=obi_row[0:1, :])

    # alpha = sigmoid(fusion_w) broadcast [128, 1]
    fsb = _tile(wprep, [1, 1], F32, 'fsb')
    nc.sync.dma_start(out=fsb[:], in_=P['fusion_w'])
    nc.scalar.activation(fsb[:], fsb[:], AF.Sigmoid)
    f16a = _tile(wprep, [1, 1], F16, 'f16a')
    nc.vector.tensor_copy(f16a[:], fsb[:])
    ps_al = _tile(wpp, [128, 1], F32, 'wp')
    nc.tensor.matmul(ps_al[:], lhsT=ones1[:], rhs=f16a[:], start=True, stop=True)
    alpha = _tile(cpool, [128, 1], F32, 'alpha')
    nc.vector.tensor_copy(alpha[:], ps_al[:])

    # nbase[p] = 800 - 3200*(p//16)  (gather window index rebase per group)
    pidq = _tile(wprep, [128, 1], I32, 'pidq')
    nc.gpsimd.iota(pidq[:], pattern=[[0, 1]], base=0, channel_multiplier=1)
    pqf = _tile(wprep, [128, 1], F32, 'pqf')
    nc.vector.tensor_copy(pqf[:], pidq[:])
    gq = _tile(wprep, [128, 1], F32, 'gq')
    nc.vector.tensor_scalar(gq[:], pqf[:], 0.5, 1.0 / 16.0, OP.add, OP.mult)
    g32 = _safe_floor(nc, wprep, gq, [128, 1], 'g32')
    nbase = _tile(cpool, [128, 1], F32, 'nbase')
    nc.vector.tensor_scalar(nbase[:], g32[:], -3200.0, 800.0, OP.mult, OP.add)

    # sigma: free-dim permutation for gather output -> linear pixel order.
    # list position j' (wrapped) should read gathered element (j'//200) +
    # 16*(j'%200); built as a wrapped i16 idx tile [128, 200].
    pb16 = _tile(wprep, [128, 1], F32, 'pb16')
    nc.vector.tensor_scalar_mul(pb16[:], g32[:], 16.0)
    jp = _tile(wprep, [128, 200], I32, 'jp')
    nc.gpsimd.iota(jp[:], pattern=[[16, 200]], base=0, channel_multiplier=1)
    jpf = _tile(wprep, [128, 200], F32, 'jpf')
    nc.vector.tensor_copy(jpf[:], jp[:])
    nc.vector.tensor_scalar(jpf[:], jpf[:], pb16[:], None, OP.subtract)
    sg_t = _tile(wprep, [128, 200], F32, 'sg_t')
    nc.vector.tensor_scalar(sg_t[:], jpf[:], 0.5, 1.0 / 200.0, OP.add, OP.mult)
    sg_fl = _safe_floor(nc, wprep, sg_t, [128, 200], 'sgf')
    sg_r = _tile(wprep, [128, 200], F32, 'sg_r')
    nc.vector.scalar_tensor_tensor(sg_r[:], in0=sg_fl[:], scalar=-200.0,
                                   in1=jpf[:], op0=OP.mult, op1=OP.add)
    nc.vector.tensor_scalar(sg_r[:], sg_r[:], 16.0, None, OP.mult)
    nc.vector.tensor_tensor(out=sg_r[:], in0=sg_r[:], in1=sg_fl[:], op=OP.add)
    sg_i = _tile(wprep, [128, 200], I32, 'sg_i')
    nc.vector.tensor_copy(sg_i[:], sg_r[:])
    sigma16 = _tile(cpool, [128, 200], I16, 'sigma16')
    nc.vector.tensor_copy(sigma16[:], sg_i[:])

    wpp.release()
    wprep.release()

    # ================= pass A =================
    spool = tc.alloc_tile_pool(name='stats', bufs=1)
    gmaxp = [_tile(spool, [128, NSLAB], F16, f'gmaxp{b}') for b in range(2)]
    colsum = [_tile(spool, [128, H, 4], F16, f'colsum{b}') for b in range(2)]

    with (tc.tile_pool(name='xslab', bufs=2) as xpool,
          tc.tile_pool(name='stp', bufs=1) as stp,
          tc.tile_pool(name='psA', bufs=2, space="PSUM") as psA):
        for s in range(NSLAB):
            xsb = [_tile(xpool, [128, SLAB], F16, f'x{b}') for b in range(2)]
            for b in range(2):
                nc.gpsimd.dma_start(out=xsb[b][:],
                                    in_=x[b * 128:(b + 1) * 128,
                                         s * SLAB:(s + 1) * SLAB])
            for mb in range(2):
                for off, wid in ((0, 2048), (2048, 2048), (4096, 1024)):
                    ps = _tile(psA, [128, 2048], F32, 'pa')
                    for j in range(0, wid, 512):
                        cs = off + j
                        for kb in range(2):
                            nc.tensor.matmul(ps[:, j:j + 512],
                                             lhsT=wT[kb][mb][:],
                                             rhs=xsb[kb][:, cs:cs + 512],
                                             start=(kb == 0), stop=(kb == 1))
                    lo = s * SLAB + off
                    nc.scalar.activation(aligned[mb][:, lo:lo + wid],
                                         ps[:, 0:wid], AF.Silu,
                                         bias=aln_bi[mb][:])
            for b in range(2):
                sl = aligned[b][:, s * SLAB:(s + 1) * SLAB]
                part = aligned[b][:].ap[0]
                tsr = aligned[b].tensor
                base = sl.offset

                def v(off, dims, _t=tsr, _p=part, _b=base):
                    return bass.AP(_t, _b + off, [_p] + dims)

                # colsum: two f16 tensor-tensor folds (2x mode), then reduce;
                # block 0's first fold runs on the otherwise-idle gpsimd
                t1 = _tile(stp, [128, 32, 4, 20], F16, 'cs1')
                eng1 = nc.vector
                with nc.allow_low_precision(reason="grid pool partial sums"):
                    eng1.tensor_tensor(
                        out=t1[:], in0=v(0, [[160, 32], [40, 4], [1, 20]]),
                        in1=v(20, [[160, 32], [40, 4], [1, 20]]), op=OP.add)
                    t1p = t1[:].ap[0]
                    t2 = _tile(stp, [128, 32, 4, 10], F16, 'cs2')
                    nc.vector.tensor_tensor(
                        out=t2[:],
                        in0=bass.AP(t1.tensor, t1[:].offset,
                                    [t1p, [80, 32], [20, 4], [1, 10]]),
                        in1=bass.AP(t1.tensor, t1[:].offset + 10,
                                    [t1p, [80, 32], [20, 4], [1, 10]]),
                        op=OP.add)
                    t2p = t2[:].ap[0]
                    t3 = _tile(stp, [128, 32, 4, 5], F16, 'cs3')
                    nc.vector.tensor_tensor(
                        out=t3[:],
                        in0=bass.AP(t2.tensor, t2[:].offset,
                                    [t2p, [40, 32], [10, 4], [1, 5]]),
                        in1=bass.AP(t2.tensor, t2[:].offset + 5,
                                    [t2p, [40, 32], [10, 4], [1, 5]]),
                        op=OP.add)
                    nc.vector.reduce_sum(
                        colsum[b][:, s * 32:(s + 1) * 32, :].rearrange(
                            "p a b -> p (a b)"),
                        t3[:], axis=AX.X)

                # gmax: f16 max-fold tree on DVE (Pool can't lower tt-max)
                g1 = _tile(stp, [128, 2560], F16, 'gm1')
                nc.vector.tensor_tensor(out=g1[:], in0=sl[:, 0:2560],
                                        in1=sl[:, 2560:5120], op=OP.max)
                g2 = _tile(stp, [128, 1280], F16, 'gm2')
                nc.vector.tensor_tensor(out=g2[:], in0=g1[:, 0:1280],
                                        in1=g1[:, 1280:2560], op=OP.max)
                g3 = _tile(stp, [128, 640], F16, 'gm3')
                nc.vector.tensor_tensor(out=g3[:], in0=g2[:, 0:640],
                                        in1=g2[:, 640:1280], op=OP.max)
                g4 = _tile(stp, [128, 320], F16, 'gm4')
                nc.vector.tensor_tensor(out=g4[:], in0=g3[:, 0:320],
                                        in1=g3[:, 320:640], op=OP.max)
                nc.vector.reduce_max(gmaxp[b][:, s:s + 1], g4[:], axis=AX.X)

    # ================= channel attention =================
    ca, rdiag = [], []
    with (tc.tile_pool(name='capool', bufs=1) as cp,
          tc.tile_pool(name='psCA', bufs=2, space="PSUM") as psCA):
        pooled, stats, locs = [], [], []
        for b in range(2):
            pl = _tile(cp, [128, 16], F32, f'pooled{b}')
            src4 = bass.AP(colsum[b].tensor, colsum[b][:].offset,
                           [colsum[b][:].ap[0], [160, 4], [1, 4], [4, 40]])
            nc.vector.reduce_sum(pl[:].rearrange("p (a b) -> p a b", a=4), src4,
                                 axis=AX.X)
            pooled.append(pl)
            st = _tile(cp, [128, 2], F16, f'stats{b}')
            tsum = _tile(cp, [128, 1], F32, f'tsum{b}')
            nc.vector.reduce_sum(tsum[:], pl[:], axis=AX.X)
            nc.vector.tensor_scalar_mul(tsum[:], tsum[:], 1.0 / HW)
            nc.vector.tensor_copy(st[:, 0:1], tsum[:])
            gm = _tile(cp, [128, 1], F32, f'gm{b}')
            nc.vector.reduce_max(gm[:], gmaxp[b][:, 0:NSLAB], axis=AX.X)
            nc.vector.tensor_copy(st[:, 1:2], gm[:])
            stats.append(st)
            lc = _tile(cp, [128, 16], F16, f'loc{b}')
            nc.vector.tensor_scalar_mul(lc[:], pl[:], 1.0 / 1600.0)
            locs.append(lc)

        def mlp2(w1T, w2T, rhs, ncol, tag):
            ps1 = _tile(psCA, [MID, ncol], F32, 'ca1')
            for b in range(2):
                nc.tensor.matmul(ps1[:], lhsT=w1T[b][:], rhs=rhs[b][:],
                                 start=(b == 0), stop=(b == 1))
            r1 = _tile(cp, [MID, ncol], F16, f'r1{tag}')
            nc.scalar.activation(r1[:], ps1[:], AF.Relu)
            outs = []
            for b in range(2):
                ps2 = _tile(psCA, [128, ncol], F32, f'ca2{b}')
                nc.tensor.matmul(ps2[:], lhsT=w2T[b][:], rhs=r1[:],
                                 start=True, stop=True)
                red = _tile(cp, [128, 1], F32, f'red{tag}{b}')
                nc.vector.reduce_sum(red[:], ps2[:], axis=AX.X)
                outs.append(red)
            return outs

        glo = mlp2(mlp_w1T, mlp_w2T, stats, 2, 'g')
        lcl = mlp2(loc_w1T, loc_w2T, locs, 16, 'l')
        for b in range(2):
            gv = _tile(cp, [128, 1], F32, f'gvec{b}')
            nc.vector.tensor_copy(gv[:], glo[b][:])
            lv = _tile(cp, [128, 1], F32, f'lvec{b}')
            nc.vector.tensor_scalar_mul(lv[:], lcl[b][:], 1.0 / 16.0)
            nc.vector.tensor_tensor(out=gv[:], in0=gv[:], in1=lv[:], op=OP.subtract)
            cab = _tile(cpool, [128, 1], F32, f'ca{b}')
            nc.vector.scalar_tensor_tensor(cab[:], in0=gv[:], scalar=alpha[:],
                                           in1=lv[:], op0=OP.mult, op1=OP.add)
            nc.scalar.activation(cab[:], cab[:], AF.Sigmoid)
            ca.append(cab)
            cv16 = _tile(cp, [128, 1], F16, f'cav{b}')
            cvf = _tile(cp, [128, 1], F32, f'cavf{b}')
            nc.vector.tensor_scalar_mul(cvf[:], cab[:], 1.0 / 256.0)
            nc.vector.tensor_copy(cv16[:], cvf[:])
            dg = _tile(cpool, [128, 129], F16, f'rdiag{b}')
            nc.vector.tensor_copy(dg[:, 0:128], ident[:])
            nc.vector.tensor_scalar_mul(dg[:, 0:128], dg[:, 0:128], cab[:])
            nc.vector.tensor_copy(dg[:, 128:129], cv16[:])
            rdiag.append(dg)
            if dbg:
                nc.sync.dma_start(out=dbg['d_ca'][b * 128:(b + 1) * 128, :], in_=cab[:])

    spool.release()

    # ================= pass B: sf (avg & max over channels) ==========
    # per 128-pixel chunk cc: transpose ca-scaled aligned with an extra
    # avg column; chunk layout tiles avgP/maxP[q, j] = pixel j*128+q.
    sfp = tc.alloc_tile_pool(name='sfpool', bufs=1)
    avgP = _tile(sfp, [128, 200], F16, 'avgP')
    maxP = _tile(sfp, [128, 200], F16, 'maxP')
    psB = tc.alloc_tile_pool(name='psB', bufs=2, space="PSUM")

    def b_batches(k0, k1):
        for k in range(k0, k1):
            ps = _tile(psB, [128, 2048], F32, 'pb')
            for j in range(4):
                cc = 4 * k + j
                for b in range(2):
                    nc.tensor.matmul(
                        ps[:, j * 512 + b * 129:j * 512 + b * 129 + 129],
                        lhsT=aligned[b][:, cc * 128:(cc + 1) * 128],
                        rhs=rdiag[b][:], start=True, stop=True)
            mview = bass.AP(ps.tensor, ps[:].offset,
                            [ps[:].ap[0], [512, 4], [129, 2], [1, 128]])
            nc.vector.tensor_reduce(maxP[:, 4 * k:4 * k + 4], mview,
                                    axis=AX.XY, op=OP.max)
            aview = bass.AP(ps.tensor, ps[:].offset + 128,
                            [ps[:].ap[0], [512, 4], [129, 2]])
            with nc.allow_low_precision(reason="sf avg f16"):
                nc.vector.reduce_sum(avgP[:, 4 * k:4 * k + 4], aview, axis=AX.X)

    def sf_assemble(half, sfT, psT):
        psa = _tile(psT, [100, 128], F16, 'ta')
        nc.tensor.transpose(out=psa[:], in_=avgP[:, half * 100:(half + 1) * 100],
                            identity=ident16[:])
        psm = _tile(psT, [100, 128], F16, 'tm')
        nc.tensor.transpose(out=psm[:], in_=maxP[:, half * 100:(half + 1) * 100],
                            identity=ident16[:])
        sfiT = _tile(sfT, [100, 128, 2], F16, 'sfiT')
        nc.vector.tensor_copy(sfiT[:, :, 0], psa[:])
        nc.vector.tensor_copy(sfiT[:, :, 1], psm[:])
        dst = bass.AP(sfe.tensor, (GT * W + half * HALF) * 2, [[1, HALF * 2]])
        nc.sync.dma_start(out=dst, in_=sfiT[:])
        sfc = [_tile(sfT, [100, 128], F16, f'sfc{chn}') for chn in range(2)]
        nc.vector.tensor_copy(sfc[0][:], psa[:])
        nc.vector.tensor_copy(sfc[1][:], psm[:])
        for chn in range(2):
            nc.scalar.dma_start(
                out=sf_cm[chn:chn + 1, half * HALF:(half + 1) * HALF],
                in_=sfc[chn][:])
            dstp = bass.AP(sf_cm_pad.tensor,
                           (PHW + 4) * chn + PW + 1 + half * 80 * PW,
                           [[PW, 80], [1, W]])
            srcc = bass.AP(sf_cm.tensor, chn * HW + half * HALF,
                           [[W, 80], [1, W]])
            nc.sync.dma_start(out=dstp, in_=srcc)
            if dbg:
                nc.scalar.dma_start(
                    out=dbg['d_sf'][chn:chn + 1, half * HALF:(half + 1) * HALF],
                    in_=sfc[chn][:])

    # conv1 + conv2: 8 interleaved slices of 20(+1) rows sharing one pool;
    # conv1 accumulates its 3 dy taps in PSUM (keeps the rhs build to one
    # contiguous dx-replicated load)
    N6S1 = 23 * PW
    NOS1 = 21 * PW
    N6S2 = 22 * PW
    NOS2 = 20 * PW

    def conv1_load(cvr, sl8):
        rbase = 20 * sl8
        nrows = 21 if sl8 < 7 else 20
        nspan = nrows * PW
        rhs6 = _tile(cvr, [6, N6S1], F16, 'rhs6')
        p6 = rhs6[:].ap[0][0]
        for dx in range(3):
            srcp = bass.AP(sf_cm_pad.tensor, rbase * PW + dx,
                           [[PHW + 4, 2], [1, (nrows + 2) * PW]])
            dst = bass.AP(rhs6.tensor, rhs6[:].offset + dx * p6,
                          [[p6 * 3, 2], [1, (nrows + 2) * PW]])
            nc.gpsimd.dma_start(out=dst, in_=srcp)
        rhs18 = _tile(cvr, [18, NOS1], F16, 'rhs18')
        for dy in range(3):
            nc.scalar.dma_start(out=rhs18[dy * 6:(dy + 1) * 6, 0:nspan],
                                in_=rhs6[:, dy * PW:dy * PW + nspan])
        return rhs18

    def conv1_slice(cvp, psC1, sl8, rhs18):
        rbase = 20 * sl8
        nrows = 21 if sl8 < 7 else 20
        nspan = nrows * PW
        o1st = _tile(cvp, [MID, NOS1], F16, 'o1st')
        for cb in range(0, nspan, 1024):
            wid = min(1024, nspan - cb)
            ps = _tile(psC1, [MID, 1024], F32, 'c1')
            for j in range(0, wid, 512):
                cw = min(512, wid - j)
                nc.tensor.matmul(ps[:, j:j + cw], lhsT=ow1T[:],
                                 rhs=rhs18[:, cb + j:cb + j + cw],
                                 start=True, stop=True)
            nc.scalar.activation(o1st[:, cb:cb + wid], ps[:, 0:wid], AF.Relu,
                                 bias=off_bi[:], scale=off_sc[:])
        po = o1st[:].ap[0][0]
        srcw = bass.AP(o1st.tensor, o1st[:].offset,
                       [[po, MID], [PW, nrows], [1, W]])
        dsto = bass.AP(o1_pad.tensor, (rbase + 1) * PW + 1,
                       [[PHW + 4, MID], [PW, nrows], [1, W]])
        nc.sync.dma_start(out=dsto, in_=srcw)

    def conv2_slice(cvp, psC2, sl8):
        rbase = 20 * sl8
        rhs48 = _tile(cvp, [48, N6S2], F16, 'rhs48')
        p48 = rhs48[:].ap[0][0]
        for dx in range(3):
            srcp = bass.AP(o1_pad.tensor, rbase * PW + dx,
                           [[PHW + 4, MID], [1, N6S2]])
            dst = bass.AP(rhs48.tensor, rhs48[:].offset + dx * p48,
                          [[p48 * 3, MID], [1, N6S2]])
            (nc.gpsimd if dx else nc.sync).dma_start(out=dst, in_=srcp)
        offst = _tile(cvp, [2, NOS2], F16, 'offst')
        for cb in range(0, NOS2, 2048):
            wid = min(2048, NOS2 - cb)
            ps = _tile(psC2, [2, 2048], F32, 'c2')
            for j in range(0, wid, 512):
                cw = min(512, wid - j)
                for dy in range(3):
                    nc.tensor.matmul(
                        ps[:, j:j + cw], lhsT=w2effT[dy][:],
                        rhs=rhs48[:, dy * PW + cb + j:dy * PW + cb + j + cw],
                        start=(dy == 0), stop=(dy == 2))
            nc.scalar.activation(offst[:, cb:cb + wid], ps[:, 0:wid],
                                 AF.Tanh, bias=b2eff[:])
        pof = offst[:].ap[0][0]
        srco = bass.AP(offst.tensor, offst[:].offset,
                       [[pof, 2], [PW, 20], [1, W]])
        dstc = bass.AP(P['off_cm'].tensor, rbase * W,
                       [[HW, 2], [W, 20], [1, W]])
        nc.sync.dma_start(out=dstc, in_=srco)
        if dbg:
            dstd = bass.AP(dbg['d_off'].tensor, rbase * W,
                           [[HW, 2], [W, 20], [1, W]])
            nc.scalar.dma_start(out=dstd, in_=srco)

    b_batches(0, 50)
    psB.release()
    with (tc.tile_pool(name='sfT', bufs=2) as sfT,
          tc.tile_pool(name='psT', bufs=2, space="PSUM") as psT):
        sf_assemble(0, sfT, psT)
        sf_assemble(1, sfT, psT)
    sfp.release()
    # fold ca into aligned now that pass B is done with the raw values;
    # the final multiply then keeps DVE fast modes (stt has none)
    for b in range(2):
        nc.vector.tensor_scalar_mul(aligned[b][:], aligned[b][:], ca[b][:])

    gk = tc.alloc_tile_pool(name='gkeep', bufs=1)
    offxA = _tile(gk, [128, 200], F16, 'offxA')
    offyA = _tile(gk, [128, 200], F16, 'offyA')
    cvr = tc.alloc_tile_pool(name='cv1r', bufs=3)
    with (tc.tile_pool(name='cv1', bufs=2) as cvp,
          tc.tile_pool(name='psC1', bufs=2, space="PSUM") as psC1):
        for s in range(8):
            rhs18 = conv1_load(cvr, s)
            conv1_slice(cvp, psC1, s, rhs18)
    cvr.release()
    with (tc.tile_pool(name='cv2', bufs=2) as cvp,
          tc.tile_pool(name='psC2', bufs=2, space="PSUM") as psC2):
        for s in range(8):
            conv2_slice(cvp, psC2, s)

    # window tiles + loads (need only sfe)
    gpoolG = tc.alloc_tile_pool(name='gatherp', bufs=1)
    gp = gpoolG
    wnd = _tile(gp, [128, WNE * 2], F16, 'wnd')
    wpitch = wnd[:].ap[0][0]
    for s in range(16):
        d0 = (0, 1, W, W + 1)[s % 4]
        srcw = bass.AP(sfe.tensor, d0 * 2, [[6400, 8], [1, WNE * 2]])
        dstw = bass.AP(wnd.tensor, wnd[:].offset + s * wpitch,
                       [[wpitch * 16, 8], [1, WNE * 2]])
        (nc.sync if s % 2 else nc.scalar).dma_start(out=dstw, in_=srcw)


    # ================= grid math =================
    with tc.tile_pool(name='gridp', bufs=1) as gq:
        for chn, t in ((0, offxA), (1, offyA)):
            srco = bass.AP(P['off_cm'].tensor, chn * HW, [[200, 128], [1, 200]])
            nc.sync.dma_start(out=t[:], in_=srco)
        pA = _tile(gq, [128, 200], I32, 'pA')
        nc.gpsimd.iota(pA[:], pattern=[[1, 200]], base=0, channel_multiplier=200)
        pf = _tile(gq, [128, 200], F32, 'pf')
        nc.vector.tensor_copy(pf[:], pA[:])
        t1 = _tile(gq, [128, 200], F32, 't1')
        nc.vector.tensor_scalar(t1[:], pf[:], 0.5, 1.0 / H, OP.add, OP.mult)
        yf = _safe_floor(nc, gq, t1, [128, 200], 'yfl')
        xf = _tile(gq, [128, 200], F32, 'xf')
        nc.vector.scalar_tensor_tensor(xf[:], in0=yf[:], scalar=-float(W), in1=pf[:],
                                       op0=OP.mult, op1=OP.add)

        def grid_axis(base_src, off_t, tag):
            u = _tile(gq, [128, 200], F32, f'u{tag}')
            of32 = _tile(gq, [128, 200], F32, f'of32{tag}')
            nc.vector.tensor_copy(of32[:], off_t[:])
            nc.vector.tensor_scalar(u[:], base_src[:], 2.0 / (W - 1), -1.0,
                                    OP.mult, OP.add)
            nc.vector.scalar_tensor_tensor(u[:], in0=of32[:], scalar=0.5, in1=u[:],
                                           op0=OP.mult, op1=OP.add)
            nc.vector.tensor_scalar(u[:], u[:], 1.0, -1.0, OP.min, OP.max)
            gc = _tile(gq, [128, 200], F32, f'g{tag}')
            nc.vector.tensor_scalar(gc[:], u[:], W / 2.0, (W - 1) / 2.0,
                                    OP.mult, OP.add)
            c0 = _safe_floor(nc, gq, gc, [128, 200], f'c0{tag}')
            wfrac = _tile(gq, [128, 200], F32, f'w{tag}')
            nc.vector.tensor_tensor(out=wfrac[:], in0=gc[:], in1=c0[:], op=OP.subtract)
            return c0, wfrac

        x0f, wx = grid_axis(xf, offxA, 'x')
        y0f, wy = grid_axis(yf, offyA, 'y')

        def mask_ts(src_t, thr, op, tag):
            m = _tile(gq, [128, 200], F32, f'm{tag}')
            nc.vector.tensor_scalar(m[:], src_t[:], thr, None, op)
            return m

        mxl = mask_ts(x0f, 0.0, OP.is_ge, 'xl')
        mxr = mask_ts(x0f, float(W - 2), OP.is_le, 'xr')
        myt = mask_ts(y0f, 0.0, OP.is_ge, 'yt')
        myb = mask_ts(y0f, float(H - 2), OP.is_le, 'yb')
        w16 = {}
        for nm, wsrc, msk, inv in (('wxl', wx, mxl, True), ('wxr', wx, mxr, False),
                                   ('wyt', wy, myt, True), ('wyb', wy, myb, False)):
            t = _tile(gq, [128, 200], F32, f'{nm}32')
            if inv:
                nc.vector.tensor_scalar(t[:], wsrc[:], -1.0, 1.0, OP.mult, OP.add)
                nc.vector.tensor_tensor(out=t[:], in0=t[:], in1=msk[:], op=OP.mult)
            else:
                nc.vector.tensor_tensor(out=t[:], in0=wsrc[:], in1=msk[:], op=OP.mult)
            h = _tile(gk, [128, 200], F16, nm)
            nc.vector.tensor_copy(h[:], t[:])
            w16[nm] = h

        # gather index: (y0f - (20g - 5)) * W + x0f  (raw floors; guards and
        # zero weights cover the out-of-range corners)
        ti = _tile(gq, [128, 200], F32, 'tif')
        nc.vector.scalar_tensor_tensor(ti[:], in0=y0f[:], scalar=float(W),
                                       in1=x0f[:], op0=OP.mult, op1=OP.add)
        nc.vector.tensor_scalar(ti[:], ti[:], nbase[:], None, OP.add)
        nc.vector.tensor_scalar(ti[:], ti[:], 0.0, float(IDX_MAX), OP.max, OP.min)
        ti32 = _tile(gq, [128, 200], I32, 'ti32')
        nc.vector.tensor_copy(ti32[:], ti[:])
        idx16 = _tile(gk, [128, 200], I16, 'idx16')
        nc.vector.tensor_copy(idx16[:], ti32[:])

    # ================= gather + bilinear =================
    samp_pad = P['samp_pad']
    if True:
        gt = _tile(gp, [128, 3200, 2], F16, 'gt')
        nc.gpsimd.ap_gather(gt[:], wnd[:].rearrange("p (n d) -> p n d", d=2),
                            idx16[:], channels=128, num_elems=WNE, d=2,
                            num_idxs=3200)
        gtP = _tile(gp, [128, 3200, 2], F16, 'gtP')
        nc.gpsimd.ap_gather(gtP[:], gt[:], sigma16[:], channels=128,
                            num_elems=3200, d=2, num_idxs=3200)
        corner = {}
        for s, nm in enumerate(('tl', 'tr', 'bl', 'br')):
            cA = _tile(gp, [128, 200, 2], F16, f'c_{nm}')
            src = bass.AP(gtP.tensor, gtP[:].offset + s * gtP[:].ap[0][0],
                          [[gtP[:].ap[0][0] * 16, 8], [1, 6400]])
            eng = (nc.scalar, nc.sync, nc.scalar, nc.sync)[s]
            eng.dma_start(out=cA[:], in_=src)
            corner[nm] = cA

        def bcast2(t):
            a = t[:].ap
            return bass.AP(t.tensor, t[:].offset, [a[0], a[1], [0, 2]])

        top = _tile(gp, [128, 200, 2], F16, 'top')
        bot = _tile(gp, [128, 200, 2], F16, 'bot')
        samp = _tile(gp, [128, 200, 2], F16, 'samp')
        tmp = _tile(gp, [128, 200, 2], F16, 'tmpc')
        nc.vector.tensor_tensor(out=top[:], in0=corner['tl'][:], in1=bcast2(w16['wxl']),
                                op=OP.mult)
        nc.vector.tensor_tensor(out=tmp[:], in0=corner['tr'][:], in1=bcast2(w16['wxr']),
                                op=OP.mult)
        nc.vector.tensor_tensor(out=top[:], in0=top[:], in1=tmp[:], op=OP.add)
        nc.vector.tensor_tensor(out=bot[:], in0=corner['bl'][:], in1=bcast2(w16['wxl']),
                                op=OP.mult)
        nc.vector.tensor_tensor(out=tmp[:], in0=corner['br'][:], in1=bcast2(w16['wxr']),
                                op=OP.mult)
        nc.vector.tensor_tensor(out=bot[:], in0=bot[:], in1=tmp[:], op=OP.add)
        nc.vector.tensor_tensor(out=top[:], in0=top[:], in1=bcast2(w16['wyt']),
                                op=OP.mult)
        nc.vector.tensor_tensor(out=bot[:], in0=bot[:], in1=bcast2(w16['wyb']),
                                op=OP.mult)
        nc.vector.tensor_tensor(out=samp[:], in0=top[:], in1=bot[:], op=OP.add)
        samp_cm = P['samp_cm']
        for chn in range(2):
            sc_t = _tile(gp, [128, 200], F16, f'sampc{chn}')
            (nc.vector if chn == 0 else nc.gpsimd).tensor_copy(
                sc_t[:], samp[:, :, chn])
            nc.scalar.dma_start(out=samp_cm[chn:chn + 1, :], in_=sc_t[:])
            dst = bass.AP(samp_pad.tensor, chn * (P7HW + 8) + 3 * P7W + 3,
                          [[P7W, H], [1, W]])
            srcc = bass.AP(samp_cm.tensor, chn * HW, [[W, H], [1, W]])
            nc.sync.dma_start(out=dst, in_=srcc)
            if dbg:
                nc.scalar.dma_start(out=dbg['d_samp'][chn:chn + 1, :],
                                    in_=sc_t[:])
    gpoolG.release()
    gk.release()

    # ================= sa + final =================
    N14Q = 46 * P7W             # conv window span per 40-row quarter
    N98Q = 40 * P7W             # packed output span per quarter
    CH2 = 2 * P7W               # 2-row chunk (332 cols incl pads)
    with (tc.tile_pool(name='rhs7', bufs=2) as rp,
          tc.tile_pool(name='outp', bufs=2) as op_,
          tc.tile_pool(name='psD', bufs=4, space="PSUM") as psD):
        for q in range(4):
            rbase = q * 40
            rhs14 = _tile(rp, [14, N14Q], F16, 'rhs14')
            p14 = rhs14[:].ap[0][0]
            for dx in range(7):
                srcp = bass.AP(samp_pad.tensor, rbase * P7W + dx,
                               [[P7HW + 8, 2], [1, N14Q]])
                dst = bass.AP(rhs14.tensor, rhs14[:].offset + dx * p14,
                              [[p14 * 7, 2], [1, N14Q]])
                (nc.scalar if dx % 2 else nc.sync).dma_start(out=dst, in_=srcp)
            rhs98 = _tile(rp, [98, N98Q], F16, 'rhs98')
            for dy in range(7):
                nc.gpsimd.dma_start(out=rhs98[dy * 14:(dy + 1) * 14, :],
                                    in_=rhs14[:, dy * P7W:dy * P7W + N98Q])
            for h2 in range(2):
                outs = [_tile(op_, [128, QTR // 2], F16, f'outs{b}')
                        for b in range(2)]
                sabh = _tile(op_, [128, 10, CH2], F16, 'sabh')
                for cp in range(5):
                    ps = _tile(psD, [128, 1024], F32, 'c7')
                    for k in range(2):
                        c = h2 * 10 + cp * 2 + k
                        nc.tensor.matmul(ps[:, k * 512:k * 512 + CH2],
                                         lhsT=attn_rep[:],
                                         rhs=rhs98[:, c * CH2:(c + 1) * CH2],
                                         start=True, stop=True)
                    pv = bass.AP(ps.tensor, ps[:].offset,
                                 [ps[:].ap[0], [512, 2], [1, CH2]])
                    nc.scalar.activation(sabh[:, 2 * cp:2 * cp + 2, :], pv,
                                         AF.Sigmoid)
                psbh = sabh[:].ap[0][0]
                sabv = bass.AP(sabh.tensor, sabh[:].offset,
                               [[psbh, 128], [CH2, 10], [P7W, 2], [1, W]])
                if dbg:
                    dstd = bass.AP(dbg['d_sa'].tensor,
                                   (rbase + h2 * 20) * W, [[W, 20], [1, W]])
                    nc.scalar.dma_start(out=dstd, in_=sabv)
                lo = q * QTR + h2 * (QTR // 2)
                for b in range(2):
                    nc.vector.tensor_tensor(
                        out=outs[b][:], in0=aligned[b][:, lo:lo + QTR // 2],
                        in1=sabv, op=OP.mult)
                    nc.sync.dma_start(out=out[b * 128:(b + 1) * 128,
                                              lo:lo + QTR // 2], in_=outs[b][:])
    apool.release()
    cpool.release()


_CACHE = {}


def _get_nc():
    if 'nc' not in _CACHE:
        _CACHE['nc'] = build(debug=False)
    return _CACHE['nc']


def prep_shared(inputs):
    """Per-core shared (replicated) weight arrays keyed by DRAM tensor name."""
    f = np.float32
    return {
        'align_w': np.ascontiguousarray(inputs['align_w'].reshape(C, C), f),
        'align_g': np.ascontiguousarray(inputs['align_g'].reshape(1, C), f),
        'align_b': np.ascontiguousarray(inputs['align_b'].reshape(1, C), f),
        'align_m': np.ascontiguousarray(inputs['align_m'].reshape(1, C), f),
        'align_v': np.ascontiguousarray(inputs['align_v'].reshape(1, C), f),
        'mlp_w1': np.ascontiguousarray(inputs['mlp_w1'].reshape(MID, C), f),
        'mlp_w2': np.ascontiguousarray(inputs['mlp_w2'].reshape(C, MID), f),
        'loc_w1': np.ascontiguousarray(inputs['loc_w1'].reshape(MID, C), f),
        'loc_w2': np.ascontiguousarray(inputs['loc_w2'].reshape(C, MID), f),
        'fusion_w': np.ascontiguousarray(
            np.asarray(inputs['fusion_w']).reshape(1, 1), f),
        'off_w1': np.ascontiguousarray(inputs['off_w1'].reshape(MID, 18), f),
        'off_g': np.ascontiguousarray(inputs['off_g'].reshape(1, MID), f),
        'off_bt': np.ascontiguousarray(inputs['off_bt'].reshape(1, MID), f),
        'off_m': np.ascontiguousarray(inputs['off_m'].reshape(1, MID), f),
        'off_v': np.ascontiguousarray(inputs['off_v'].reshape(1, MID), f),
        'off_w2': np.ascontiguousarray(inputs['off_w2'].reshape(98, 144), f),
        'off_b2': np.ascontiguousarray(inputs['off_b2'].reshape(1, 98), f),
        'attn_w': np.ascontiguousarray(inputs['attn_w'].reshape(1, 98), f),
    }


def _get_runner():
    """Cached jitted shard_map runner (avoids re-tracing per call)."""
    if 'runner' in _CACHE:
        return _CACHE['runner']
    import jax
    import jax.numpy as jnp
    from jax.experimental.shard_map import shard_map
    from jax.sharding import Mesh, NamedSharding, PartitionSpec
    from concourse import bass2jax
    import concourse.mybir as mb

    nc = _get_nc()
    bass2jax.install_neuronx_cc_hook()
    part_name = nc.partition_id_tensor.name if nc.partition_id_tensor else None
    in_names, out_names, out_shapes = [], [], []
    for alloc in nc.m.functions[0].allocations:
        if not isinstance(alloc, mb.MemoryLocationSet):
            continue
        name = alloc.memorylocations[0].name
        if alloc.kind == "ExternalInput":
            if name != part_name:
                in_names.append(name)
        elif alloc.kind == "ExternalOutput":
            out_names.append(name)
            out_shapes.append((tuple(alloc.tensor_shape), mb.dt.np(alloc.dtype)))
    n_params = len(in_names)
    n_outs = len(out_names)
    out_avals = tuple(jax.core.ShapedArray(s, d) for s, d in out_shapes)
    all_in = list(in_names) + list(out_names)
    if part_name is not None:
        all_in.append(part_name)

    def _body(*args):
        operands = list(args)
        if part_name is not None:
            operands.append(bass2jax.partition_id_tensor())
        outs = bass2jax._bass_exec_p.bind(
            *operands, out_avals=out_avals, in_names=tuple(all_in),
            out_names=tuple(out_names), lowering_input_output_aliases=(),
            sim_require_finite=True, sim_require_nnan=True, nc=nc)
        return tuple(outs)

    devices = jax.devices()[:B]
    mesh = Mesh(np.asarray(devices), ("core",))
    spec = PartitionSpec("core")
    sharded = jax.jit(
        shard_map(_body, mesh=mesh, in_specs=(spec,) * (n_params + n_outs),
                  out_specs=(spec,) * n_outs, check_rep=False),
        donate_argnums=tuple(range(n_params, n_params + n_outs)),
        keep_unused=True)
    zero_fns = [
        jax.jit(lambda s=s, d=d: jnp.zeros((B * s[0],) + s[1:], d),
                out_shardings=NamedSharding(mesh, spec))
        for s, d in out_shapes]
    _CACHE['runner'] = (sharded, zero_fns, in_names, out_names, mesh, spec)
    return _CACHE['runner']


def kernel(**inputs):
    shared = prep_shared(inputs)
    xs = np.asarray(inputs['x']).reshape(B * C, HW).astype(np.float16)
    sharded, zero_fns, in_names, out_names, mesh, spec = _get_runner()
    concat = {k: np.concatenate([v] * B, axis=0) for k, v in shared.items()}
    concat['x'] = xs
    args = [concat[n] for n in in_names]
    zeros = [zf() for zf in zero_fns]
    outs = sharded(*args, *zeros)
    oi = out_names.index('out')
    return np.asarray(outs[oi]).astype(np.float32).reshape(B, C, H, W)

